# revision 44
# baseline (speedup 1.0000x reference)
"""Multi-head attention (B=2, S=2048, D=1024, H=16) on 8 trn2 NeuronCores.

Sharding: core c -> batch b = c//4, head-group g = c%4 (4 heads each).
Tensor-parallel on heads: each core projects q/k/v for its 4 heads only
(column-sharded W_q/W_k/W_v), runs full-sequence attention for those heads,
all-gathers the per-head attention outputs within its 4-core batch group,
and computes a 256-column slice of the final W_o projection. The host
reassembles the [2, 2048, 1024] output from the 8 per-core [256, 2048]
(transposed) slices.

v2 structure (vs the ~287us phase-separated predecessor; ~275us in the
same fabric conditions where v1 measures ~300):
- The exp stream (128x [128,1024] ACTIVATEs, 142.5us) now starts at
  ~35us instead of ~82us: X is host-swizzled to [128, qb*8+e, 512] so
  each seq-quarter chunk is one contiguous-8KB-per-partition DMA, and
  the projection chains (8 e-tile matmuls + DVE eviction each) chase
  the chunks, spliced between attention units in PE emission order.
- ~90 dummy matmuls on never-written SBUF fire at t~0 to hold the HAM
  clock-gate open, so the first real chains run at full clock.
- Attention unit order is quarter-outer / pair-inner. One AllGather
  per quarter for q0-2 (pairs concatenated on the free axis -- the CC
  core serializes ops from BOTH replica groups, so fewer mid-stream
  ops keep it unsaturated); quarter 3 is split per-pair so its p0
  piece gathers under the last block and only a 128KB op remains
  exposed in the tail.
- finish() copies the PV PSUM to SBUF first (banks free in ~1.5us
  instead of locked behind the whole normalization chain), which
  killed a HAM re-throttle oscillation at every block boundary.
- ALL O-projection chains run in the tail: quarters 0-2 (AGs long
  done) fill the last AG's service window; quarter 3's pair-0 halves
  run before it, pair-1 halves after. Output is written bf16.
- PSUM: scores ping-pong 2x[128,1024] (4 banks) + PV accumulators
  2x[65,512] single-generation (2 banks) + a 2-bank ping-pong pool
  for projection chains / warm-up / q3 O-chains; q0-2 O-chains reuse
  the PV pools.
- Everything else (head-pair packing, ones-column softmax sums, DVE
  normalization, bias foldings, gpsimd AG prefetch, warm-up collective
  at t~0) is inherited from v1.
"""

import os

import ml_dtypes
import numpy as np

import concourse.bacc as bacc
import concourse.mybir as mybir
import concourse.tile as tile
from concourse import bass_utils

B, S, D, H = 2, 2048, 1024, 16
Dh = D // H  # 64
N_CORES = 8
HL = H // 4  # heads per core (4)
DL = HL * Dh  # local head dims (256)
P = 128
E_TILES = D // P  # 8
KT = S // P  # 16 key tiles
QQ = 512  # s_q quarter block

f32 = mybir.dt.float32
bf16 = mybir.dt.bfloat16
AF = mybir.ActivationFunctionType

TRACE = False  # test harness sets kernel.TRACE = True for profiling


def _build():
    nc = bacc.Bacc("TRN2", target_bir_lowering=False, debug=False,
                   num_devices=N_CORES)

    # X swizzled [128, qb*8+e, 512]: one seq-quarter = contiguous 8KB/part
    XvS = nc.dram_tensor("XvS", [P, 32, QQ], bf16, kind="ExternalInput").ap()
    XkS = nc.dram_tensor("XkS", [P, 32, QQ], bf16, kind="ExternalInput").ap()
    XqS = nc.dram_tensor("XqS", [P, 32, QQ], bf16, kind="ExternalInput").ap()
    WvS = nc.dram_tensor("WvS", [P, E_TILES, DL], bf16,
                         kind="ExternalInput").ap()
    WkS = nc.dram_tensor("WkS", [P, E_TILES, DL], bf16,
                         kind="ExternalInput").ap()
    WqS = nc.dram_tensor("WqS", [P, E_TILES, DL], bf16,
                         kind="ExternalInput").ap()
    WoS = nc.dram_tensor("WoS", [P, E_TILES, DL], bf16,
                         kind="ExternalInput").ap()
    bqko = nc.dram_tensor("bqko", [P, 6], f32, kind="ExternalInput").ap()
    out = nc.dram_tensor("out", [DL, S], bf16, kind="ExternalOutput").ap()

    with tile.TileContext(nc) as tc:
        with (
            tc.tile_pool(name="const", bufs=1) as cp,
            tc.tile_pool(name="qk", bufs=1) as qkp,
            tc.tile_pool(name="vt", bufs=1) as vtp,
            tc.tile_pool(name="xs", bufs=1) as xsp,
            tc.tile_pool(name="rhs", bufs=1) as rhp,
            tc.tile_pool(name="exp", bufs=6) as expp,
            tc.tile_pool(name="nrm", bufs=1) as nrmp,
            tc.tile_pool(name="ao", bufs=4) as aop,
            tc.tile_pool(name="res", bufs=2) as resp,
            tc.tile_pool(name="dram", bufs=1, space="DRAM") as dram,
        ):
            # ---- SBUF tiles ----
            wv = cp.tile([P, E_TILES, DL], bf16, tag="wv")
            wk = cp.tile([P, E_TILES, DL], bf16, tag="wk")
            wq = cp.tile([P, E_TILES, DL], bf16, tag="wq")
            wo = cp.tile([P, E_TILES, DL], bf16, tag="wo")
            bias_c = cp.tile([P, 6], f32, tag="bias")
            bq_c = bias_c[:, 0:2]
            bk_c = bias_c[:, 2:4]
            bo_c = bias_c[:, 4:6]

            xv = xsp.tile([P, 32, QQ], bf16, tag="xv", name="xv")
            xk = xsp.tile([P, 32, QQ], bf16, tag="xk", name="xk")
            xq = xsp.tile([P, 32, QQ], bf16, tag="xq", name="xq")

            v_sb = vtp.tile([P, KT, HL, Dh + 1], bf16)
            kT = [qkp.tile([P, S], bf16, tag=f"kT{i}", name=f"kT{i}")
                  for i in range(2)]
            qT = [qkp.tile([P, S], bf16, tag=f"qT{i}", name=f"qT{i}")
                  for i in range(2)]
            rhs_sb = rhp.tile([P, 4, E_TILES, QQ], bf16)

            # warm-up collective FIRST on the idle gpsimd queue: fires at
            # t~0 on garbage dram data (output unused) so the one-time
            # ncfw setup cost finishes before the first real AllGather.
            wagi = dram.tile([P, 4], bf16, tag="wagi", name="wagi")
            wago = dram.tile([4 * P, 4], bf16, tag="wago", name="wago")
            nc.gpsimd.collective_compute(
                "AllGather",
                mybir.AluOpType.bypass,
                replica_groups=[[0, 1, 2, 3], [4, 5, 6, 7]],
                ins=[wagi.opt()],
                outs=[wago.opt()],
            )

            # ---- DMA issue (all up front, priority order) ----
            # ring A (sync, fast): weights first (tiny), then X chunks in
            # consumption order. ring B (scalar): late-deadline Xq
            # quarters, v q3, wo. (Loading ring B heavily slows the
            # collectives -- SDMA contention -- so it stays light.)
            nc.sync.dma_start(wk[:], WkS)
            nc.sync.dma_start(xk[:, 0:8, :], XkS[:, 0:8, :])
            nc.sync.dma_start(wv[:], WvS)
            nc.sync.dma_start(xv[:, 0:8, :], XvS[:, 0:8, :])
            nc.sync.dma_start(xk[:, 8:16, :], XkS[:, 8:16, :])
            nc.sync.dma_start(xv[:, 8:16, :], XvS[:, 8:16, :])
            nc.sync.dma_start(xv[:, 16:24, :], XvS[:, 16:24, :])
            nc.sync.dma_start(xv[:, 24:32, :], XvS[:, 24:32, :])

            nc.scalar.dma_start(wq[:], WqS)
            nc.scalar.dma_start(bias_c[:], bqko)
            nc.scalar.dma_start(xq[:, 0:8, :], XqS[:, 0:8, :])
            nc.scalar.dma_start(xk[:, 16:24, :], XkS[:, 16:24, :])
            nc.scalar.dma_start(xk[:, 24:32, :], XkS[:, 24:32, :])
            nc.scalar.dma_start(wo[:], WoS)
            nc.scalar.dma_start(xq[:, 8:16, :], XqS[:, 8:16, :])
            nc.scalar.dma_start(xq[:, 16:24, :], XqS[:, 16:24, :])
            nc.scalar.dma_start(xq[:, 24:32, :], XqS[:, 24:32, :])

            # preload the exp table while DMAs stream
            warm = cp.tile([P, 1], f32, tag="warm")
            nc.gpsimd.memset(warm[:], 0.0)
            nc.scalar.activation(warm[:], warm[:], AF.Exp)

            nc.vector.memset(v_sb[:, :, :, Dh], 1.0)

            # one AllGather per quarter for q0-2 (both pairs concatenated
            # on the free axis): the CC core serializes ops from BOTH
            # replica groups, so fewer mid-stream ops keep it unsaturated
            # even on a slow fabric. Quarter 3 is split per-pair: its p0
            # piece gathers under the last block, so only a 128KB op
            # (smallest possible service) remains exposed in the tail.
            ag_in = [dram.tile([P, 2 * QQ], bf16, tag=f"agi{qq}",
                               name=f"agi{qq}") for qq in range(3)]
            ag_out = [dram.tile([4 * P, 2 * QQ], bf16, tag=f"ago{qq}",
                                name=f"ago{qq}") for qq in range(3)]
            ag3_in = [dram.tile([P, QQ], bf16, tag=f"agi3{p}",
                                name=f"agi3{p}") for p in range(2)]
            ag3_out = [dram.tile([4 * P, QQ], bf16, tag=f"ago3{p}",
                                 name=f"ago3{p}") for p in range(2)]

            with (
                tc.tile_pool(name="pss", bufs=2, space="PSUM") as pssp,
                tc.tile_pool(name="psoj0", bufs=1, space="PSUM") as psoj0p,
                tc.tile_pool(name="psoj1", bufs=1, space="PSUM") as psoj1p,
                tc.tile_pool(name="pwork", bufs=2, space="PSUM") as pwp,
            ):
                # ---- chain emitters (projections / O-projection) ----
                def kq_chain(which, p, qb):
                    """Project pair p, seq-quarter qb of k or q."""
                    w, x, dst, b_c = (
                        (wk, xk, kT, bk_c) if which == "k"
                        else (wq, xq, qT, bq_c)
                    )
                    pw = pwp.tile([P, QQ], f32, tag="pw",
                                  name=f"p{which}{p}{qb}")
                    for e in range(E_TILES):
                        nc.tensor.matmul(
                            pw[:],
                            w[:, e, p * P:(p + 1) * P],
                            x[:, qb * 8 + e, :],
                            start=(e == 0),
                            stop=(e == E_TILES - 1),
                        )
                    nc.vector.tensor_scalar_add(
                        dst[p][:, qb * QQ:(qb + 1) * QQ], pw[:],
                        b_c[:, p:p + 1],
                    )

                def v_chain(kt):
                    """Project v key-tile kt into natural [s,h,d] layout."""
                    pw = pwp.tile([P, QQ], f32, tag="pw", name=f"pv{kt}")
                    qb, kk = kt // 4, kt % 4
                    for e in range(E_TILES):
                        nc.tensor.matmul(
                            pw[:, 0:DL],
                            xv[:, qb * 8 + e, kk * P:(kk + 1) * P],
                            wv[:, e, :],
                            start=(e == 0),
                            stop=(e == E_TILES - 1),
                        )
                    nc.vector.tensor_copy(
                        v_sb[:, kt, :, 0:Dh],
                        pw[:, 0:DL].rearrange("p (h d) -> p h d", h=HL),
                    )

                eorder = (0, 2, 4, 6, 1, 3, 5, 7)  # pair-0 ranks first

                def o_chain(qq, dt, pool, tag):
                    """O-projection slice [dt] for seq-quarter qq."""
                    po = pool.tile([P, QQ], f32, tag=tag, name=f"po{qq}{dt}")
                    for ei, e in enumerate(eorder):
                        nc.tensor.matmul(
                            po[:],
                            wo[:, e, dt * P:(dt + 1) * P],
                            rhs_sb[:, qq, e, :],
                            start=(ei == 0),
                            stop=(ei == E_TILES - 1),
                        )
                    ot = resp.tile([P, QQ], bf16, tag="ot", name="ot")
                    nc.scalar.activation(
                        ot[:], po[:], AF.Identity,
                        bias=bo_c[:, dt:dt + 1], scale=1.0,
                    )
                    nc.sync.dma_start(
                        out[dt * P:(dt + 1) * P, qq * QQ:(qq + 1) * QQ],
                        ot[:],
                    )

                # ---- attention: (quarter, pair) blocks x key-tiles ----
                pso_cur = {}

                def scores(u):
                    qq, p, kt = u
                    pss = pssp.tile([P, 2 * QQ], f32, tag="pss", name="pss")
                    for j in range(2):
                        nc.tensor.matmul(
                            pss[:, j * QQ:(j + 1) * QQ],
                            kT[p][64 * j:64 * j + Dh, kt * P:(kt + 1) * P],
                            qT[p][64 * j:64 * j + Dh,
                                  qq * QQ:(qq + 1) * QQ],
                            start=True,
                            stop=True,
                        )
                    ex = expp.tile([P, 2 * QQ], bf16, tag="ex", name="ex")
                    nc.scalar.activation(ex[:], pss[:], AF.Exp)
                    return ex

                def finish(qq, p, pso):
                    # copy PSUM out to SBUF first: the pso banks free after
                    # ~1.5us instead of staying locked for the whole
                    # normalization chain (which would micro-idle the PE at
                    # the block boundary and trip the HAM re-throttle).
                    po_sb = []
                    for j in range(2):
                        ps = nrmp.tile([Dh + 1, QQ], f32, tag=f"posb{j}",
                                       name=f"posb{j}")
                        nc.vector.tensor_copy(ps[:], pso[j][:])
                        po_sb.append(ps)
                    for j in range(2):
                        sums = nrmp.tile([1, QQ], f32, tag="sm", name="sm")
                        nc.sync.dma_start(sums[:], po_sb[j][Dh:Dh + 1, :])
                        recip = nrmp.tile([1, QQ], f32, tag="rc", name="rc")
                        nc.vector.reciprocal_approx_fast(recip[:], sums[:])
                        rcb = nrmp.tile([64, QQ], f32, tag="rcb", name="rcb")
                        nc.sync.dma_start(rcb[0:1, :], recip[:])
                        nc.sync.dma_start(rcb[32:33, :], recip[:])
                        bc = nrmp.tile([64, QQ], f32, tag="bc", name="bc")
                        nc.vector.stream_shuffle(bc[:], rcb[:],
                                                 mask=[0] * 32)
                        ao = aop.tile([Dh, QQ], bf16, tag="ao", name="ao")
                        nc.vector.tensor_mul(ao[:], po_sb[j][0:Dh, :], bc[:])
                        if qq < 3:
                            nc.sync.dma_start(
                                ag_in[qq][64 * j:64 * j + Dh,
                                           p * QQ:(p + 1) * QQ],
                                ao[:],
                            )
                        else:
                            nc.sync.dma_start(
                                ag3_in[p][64 * j:64 * j + Dh, :], ao[:]
                            )
                    if qq < 3 and p == 1:
                        nc.gpsimd.collective_compute(
                            "AllGather",
                            mybir.AluOpType.bypass,
                            replica_groups=[[0, 1, 2, 3], [4, 5, 6, 7]],
                            ins=[ag_in[qq].opt()],
                            outs=[ag_out[qq].opt()],
                        )
                        # prefetch on the gpsimd queue (parked on CC
                        # anyway, so the CC-gated triggers stall nothing):
                        # rank r's [128, 1024] row-block lands in e-slots
                        # 2r (pair 0) and 2r+1 (pair 1).
                        for r in range(4):
                            nc.gpsimd.dma_start(
                                rhs_sb[:, qq, 2 * r:2 * r + 2, :],
                                ag_out[qq][r * P:(r + 1) * P, :],
                            )
                    elif qq == 3:
                        nc.gpsimd.collective_compute(
                            "AllGather",
                            mybir.AluOpType.bypass,
                            replica_groups=[[0, 1, 2, 3], [4, 5, 6, 7]],
                            ins=[ag3_in[p].opt()],
                            outs=[ag3_out[p].opt()],
                        )
                        # the last piece prefetches on the now-idle sync
                        # hwdge ring, which is faster than swdge.
                        eng = nc.gpsimd if p == 0 else nc.sync
                        for r in range(4):
                            eng.dma_start(
                                rhs_sb[:, 3, 2 * r + p, :],
                                ag3_out[p][r * P:(r + 1) * P, :],
                            )

                def pv(u, ex):
                    qq, p, kt = u
                    if kt == 0:
                        pso_cur[(qq, p)] = [
                            psoj0p.tile([Dh + 1, QQ], f32, tag="pso0",
                                        name="pso0"),
                            psoj1p.tile([Dh + 1, QQ], f32, tag="pso1",
                                        name="pso1"),
                        ]
                    pso = pso_cur[(qq, p)]
                    for j in range(2):
                        nc.tensor.matmul(
                            pso[j][:],
                            v_sb[:, kt, 2 * p + j, :],
                            ex[:, j * QQ:(j + 1) * QQ],
                            start=(kt == 0),
                            stop=(kt == KT - 1),
                        )
                    if kt == KT - 1:
                        finish(qq, p, pso)

                units = [
                    (qq, p, kt)
                    for qq in range(4)
                    for p in range(2)
                    for kt in range(KT)
                ]

                # chain ops spliced into the PE stream after unit i.
                # V(kt) at unit kt: maximally late (just before its PV at
                # unit kt+1) so a late v chunk never head-of-line blocks
                # the scores/exp stream. K(p,qb) before scores kt=4qb.
                # O(qq) ~1.5 blocks after AG(qq,p1) fires.
                after_unit = {
                    0: [("v", 0)],
                    1: [("k", 0, 1), ("v", 1)],
                    2: [("v", 2)], 3: [("v", 3)],
                    4: [("v", 4)], 5: [("k", 0, 2), ("v", 5)],
                    6: [("v", 6)], 7: [("v", 7)],
                    8: [("q", 1, 0), ("v", 8)],
                    9: [("k", 0, 3), ("v", 9)],
                    10: [("v", 10)], 11: [("k", 1, 1), ("v", 11)],
                    12: [("v", 12)], 13: [("k", 1, 2), ("v", 13)],
                    14: [("v", 14)],
                    15: [("k", 1, 3), ("v", 15)],
                    28: [("q", 0, 1)],
                    44: [("q", 1, 1)],
                    58: [("q", 0, 2)],
                    76: [("q", 1, 2)],
                    90: [("q", 0, 3)],
                    108: [("q", 1, 3)],
                }

                def emit_chain(op):
                    if op[0] == "v":
                        v_chain(op[1])
                    else:
                        kq_chain(op[0], op[1], op[2])

                # PE clock warm-up: ~90 dummy matmuls on never-written
                # SBUF (no data deps, so they fire at t~0 while the DMAs
                # stream). The HAM un-throttles after ~3.4us of activity,
                # so the real projection chains at ~20us run at full
                # clock instead of the cold 4/8 default.
                junk = cp.tile([P, QQ], bf16, tag="junk")
                nc.vector.memset(junk[:], 0.0)
                wmm = pwp.tile([P, QQ], f32, tag="pw", name="wmm")
                for _ in range(90):
                    nc.tensor.matmul(wmm[:], junk[:, 0:P], junk[:],
                                     start=True, stop=True)

                # prologue: only what unit 0 needs (pair-0 k and q for
                # the first key/query quarters) plus K(1,0) as PE filler;
                # Q(1,0) rides the stream (needed by unit 16).
                kq_chain("k", 0, 0)
                kq_chain("q", 0, 0)
                kq_chain("k", 1, 0)

                ex_prev = scores(units[0])
                for op in after_unit.get(0, []):
                    emit_chain(op)
                for i in range(1, len(units)):
                    ex = scores(units[i])
                    pv(units[i - 1], ex_prev)
                    ex_prev = ex
                    for op in after_unit.get(i, []):
                        emit_chain(op)
                pv(units[-1], ex_prev)

                # tail: ALL O-projections run here, ordered so the PE is
                # never idle during the last gather. Quarter 3's pair-0
                # halves first (their AG completed under block 7), then
                # quarters 0-2 (AGs long done) fill the final 128KB AG's
                # service window, then quarter 3's pair-1 halves.
                po3 = [pwp.tile([P, QQ], f32, tag="pw", name=f"po3{dt}")
                       for dt in range(2)]
                for dt in range(2):
                    for ei, e in enumerate((0, 2, 4, 6)):
                        nc.tensor.matmul(
                            po3[dt][:],
                            wo[:, e, dt * P:(dt + 1) * P],
                            rhs_sb[:, 3, e, :],
                            start=(ei == 0),
                            stop=False,
                        )
                for qq in range(3):
                    o_chain(qq, 0, psoj0p, "pso0")
                    o_chain(qq, 1, psoj1p, "pso1")
                for dt in range(2):
                    for ei, e in enumerate((1, 3, 5, 7)):
                        nc.tensor.matmul(
                            po3[dt][:],
                            wo[:, e, dt * P:(dt + 1) * P],
                            rhs_sb[:, 3, e, :],
                            start=False,
                            stop=(ei == 3),
                        )
                for dt in range(2):
                    ot = resp.tile([P, QQ], bf16, tag="ot", name="ot")
                    nc.scalar.activation(
                        ot[:], po3[dt][:], AF.Identity,
                        bias=bo_c[:, dt:dt + 1], scale=1.0,
                    )
                    nc.sync.dma_start(
                        out[dt * P:(dt + 1) * P, 3 * QQ:4 * QQ], ot[:],
                    )

    nc.compile()
    return nc


def kernel(**inputs):
    Q = np.asarray(inputs["Q"], dtype=np.float32)
    K = np.asarray(inputs["K"], dtype=np.float32)
    V = np.asarray(inputs["V"], dtype=np.float32)
    Wq = np.asarray(inputs["Wq"], dtype=np.float32)
    Wk = np.asarray(inputs["Wk"], dtype=np.float32)
    Wv = np.asarray(inputs["Wv"], dtype=np.float32)
    Wo = np.asarray(inputs["Wo"], dtype=np.float32)
    bq = np.asarray(inputs["bq"], dtype=np.float32)
    bk = np.asarray(inputs["bk"], dtype=np.float32)
    bv = np.asarray(inputs["bv"], dtype=np.float32)
    bo = np.asarray(inputs["bo"], dtype=np.float32)

    nc = _build()

    def xswz(x):  # [S, D] -> [128, qb*8+e, 512] (contiguous quarter chunks)
        t = x.T.reshape(E_TILES, P, 4, QQ).transpose(1, 2, 0, 3)
        return np.ascontiguousarray(t.reshape(P, 32, QQ)).astype(
            ml_dtypes.bfloat16)

    XT = {
        b: {
            "XvS": xswz(V[b]),
            "XkS": xswz(K[b]),
            "XqS": xswz(Q[b]),
        }
        for b in range(B)
    }

    # Heads within a group are laid out (0, 2, 1, 3) so each projected tile
    # holds a head pair spanning both PE row-group halves. W tensors are
    # pre-swizzled to the SBUF layout [128, e-tile, 256] for contiguous DMA.
    horder = (0, 2, 1, 3)

    def swz(w):  # [256 out-perm, 1024 in] -> [128, 8, 256]
        return np.ascontiguousarray(
            w.T.reshape(E_TILES, P, DL).transpose(1, 0, 2)
        ).astype(ml_dtypes.bfloat16)

    # Wo columns per e-tile follow the gathered attention rows:
    # e = (rank r, pair p); rows = heads horder[2p], horder[2p+1] of rank r.
    colperm = []
    for r in range(4):
        for p_ in range(2):
            for j in range(2):
                hl = horder[2 * p_ + j]
                colperm.extend(range(DL * r + 64 * hl, DL * r + 64 * hl + 64))
    colperm = np.array(colperm)

    Wslices = {}
    for g in range(4):
        idx = np.concatenate(
            [
                np.arange(DL * g + 64 * hl, DL * g + 64 * hl + 64)
                for hl in horder
            ]
        )
        rows = slice(DL * g, DL * (g + 1))
        Wslices[g] = {
            "WvS": swz(Wv[idx]),
            "WkS": swz(Wk[idx]),
            "WqS": swz(Wq[idx] * 0.125),
            "WoS": swz(Wo[rows][:, colperm]),
            "bqko": np.ascontiguousarray(
                np.stack(
                    [
                        bq[idx] * 0.125,
                        bk[idx],
                        bo[rows] + Wo[rows] @ bv,
                    ],
                    axis=0,
                )
                .reshape(3, 2, P)
                .transpose(2, 0, 1)
                .reshape(P, 6)
            ).astype(np.float32),
        }

    in_maps = []
    for c in range(N_CORES):
        b, g = c // 4, c % 4
        m = dict(XT[b])
        m.update(Wslices[g])
        in_maps.append(m)

    trace_cores = (
        list(range(N_CORES)) if os.environ.get("TRACE_ALL") else None
    )
    res = bass_utils.run_bass_kernel_spmd(
        nc, in_maps, core_ids=list(range(N_CORES)), trace=TRACE,
        trace_cores=trace_cores,
    )

    full = np.empty((B, S, D), dtype=np.float32)
    for c in range(N_CORES):
        b, g = c // 4, c % 4
        full[b, :, DL * g:DL * (g + 1)] = (
            res.results[c]["out"].astype(np.float32).T
        )
    if TRACE:
        kernel.last_result = res
    return full


kernel.last_result = None


# revision 46
# speedup vs baseline: 1.1900x; 1.1900x over previous
"""Multi-head attention (B=2, S=2048, D=1024, H=16) on 8 trn2 NeuronCores.

Sharding: core c -> batch b = c//4, head-group g = c%4 (4 heads each).
Tensor-parallel on heads: each core projects q/k/v for its 4 heads only
(column-sharded W_q/W_k/W_v), runs full-sequence attention for those heads,
all-gathers the per-head attention outputs within its 4-core batch group,
and computes a 256-column slice of the final W_o projection. The host
reassembles the [2, 2048, 1024] output from the 8 per-core [256, 2048]
(transposed) slices.

v2 structure (vs the ~287us phase-separated predecessor; ~275us in the
same fabric conditions where v1 measures ~300):
- The exp stream (128x [128,1024] ACTIVATEs, 142.5us) now starts at
  ~35us instead of ~82us: X is host-swizzled to [128, qb*8+e, 512] so
  each seq-quarter chunk is one contiguous-8KB-per-partition DMA, and
  the projection chains (8 e-tile matmuls + DVE eviction each) chase
  the chunks, spliced between attention units in PE emission order.
- ~90 dummy matmuls on never-written SBUF fire at t~0 to hold the HAM
  clock-gate open, so the first real chains run at full clock.
- Attention unit order is quarter-outer / pair-inner. One AllGather
  per quarter for q0-2 (pairs concatenated on the free axis -- the CC
  core serializes ops from BOTH replica groups, so fewer mid-stream
  ops keep it unsaturated); quarter 3 is split per-pair so its p0
  piece gathers under the last block and only a 128KB op remains
  exposed in the tail.
- finish() copies the PV PSUM to SBUF first (banks free in ~1.5us
  instead of locked behind the whole normalization chain), which
  killed a HAM re-throttle oscillation at every block boundary.
- ALL O-projection chains run in the tail: quarters 0-2 (AGs long
  done) fill the last AG's service window; quarter 3's pair-0 halves
  run before it, pair-1 halves after. Output is written bf16.
- PSUM: scores ping-pong 2x[128,1024] (4 banks) + PV accumulators
  2x[65,512] single-generation (2 banks) + a 2-bank ping-pong pool
  for projection chains / warm-up / q3 O-chains; q0-2 O-chains reuse
  the PV pools.
- Everything else (head-pair packing, ones-column softmax sums, DVE
  normalization, bias foldings, gpsimd AG prefetch, warm-up collective
  at t~0) is inherited from v1.
"""

import os

import ml_dtypes
import numpy as np

import concourse.bacc as bacc
import concourse.mybir as mybir
import concourse.tile as tile
from concourse import bass_utils

B, S, D, H = 2, 2048, 1024, 16
Dh = D // H  # 64
N_CORES = 8
HL = H // 4  # heads per core (4)
DL = HL * Dh  # local head dims (256)
P = 128
E_TILES = D // P  # 8
KT = S // P  # 16 key tiles
QQ = 512  # s_q quarter block

f32 = mybir.dt.float32
bf16 = mybir.dt.bfloat16
AF = mybir.ActivationFunctionType

TRACE = False  # test harness sets kernel.TRACE = True for profiling


def _build():
    nc = bacc.Bacc("TRN2", target_bir_lowering=False, debug=False,
                   num_devices=N_CORES)

    # X swizzled [128, qb*8+e, 512]: one seq-quarter = contiguous 8KB/part
    XvS = nc.dram_tensor("XvS", [P, 32, QQ], bf16, kind="ExternalInput").ap()
    XkS = nc.dram_tensor("XkS", [P, 32, QQ], bf16, kind="ExternalInput").ap()
    XqS = nc.dram_tensor("XqS", [P, 32, QQ], bf16, kind="ExternalInput").ap()
    WvS = nc.dram_tensor("WvS", [P, E_TILES, DL], bf16,
                         kind="ExternalInput").ap()
    WkS = nc.dram_tensor("WkS", [P, E_TILES, DL], bf16,
                         kind="ExternalInput").ap()
    WqS = nc.dram_tensor("WqS", [P, E_TILES, DL], bf16,
                         kind="ExternalInput").ap()
    WoS = nc.dram_tensor("WoS", [P, E_TILES, DL], bf16,
                         kind="ExternalInput").ap()
    bqko = nc.dram_tensor("bqko", [P, 6], f32, kind="ExternalInput").ap()
    out = nc.dram_tensor("out", [DL, S], bf16, kind="ExternalOutput").ap()

    with tile.TileContext(nc) as tc:
        with (
            tc.tile_pool(name="const", bufs=1) as cp,
            tc.tile_pool(name="qk", bufs=1) as qkp,
            tc.tile_pool(name="vt", bufs=1) as vtp,
            tc.tile_pool(name="xs", bufs=1) as xsp,
            tc.tile_pool(name="rhs", bufs=1) as rhp,
            tc.tile_pool(name="exp", bufs=6) as expp,
            tc.tile_pool(name="nrm", bufs=1) as nrmp,
            tc.tile_pool(name="ao", bufs=4) as aop,
            tc.tile_pool(name="res", bufs=2) as resp,
            tc.tile_pool(name="dram", bufs=1, space="DRAM") as dram,
        ):
            # ---- SBUF tiles ----
            wv = cp.tile([P, E_TILES, DL], bf16, tag="wv")
            wk = cp.tile([P, E_TILES, DL], bf16, tag="wk")
            wq = cp.tile([P, E_TILES, DL], bf16, tag="wq")
            wo = cp.tile([P, E_TILES, DL], bf16, tag="wo")
            bias_c = cp.tile([P, 6], f32, tag="bias")
            bq_c = bias_c[:, 0:2]
            bk_c = bias_c[:, 2:4]
            bo_c = bias_c[:, 4:6]

            xv = xsp.tile([P, 32, QQ], bf16, tag="xv", name="xv")
            xk = xsp.tile([P, 32, QQ], bf16, tag="xk", name="xk")
            xq = xsp.tile([P, 32, QQ], bf16, tag="xq", name="xq")

            v_sb = vtp.tile([P, KT, HL, Dh + 1], bf16)
            kT = [qkp.tile([P, S], bf16, tag=f"kT{i}", name=f"kT{i}")
                  for i in range(2)]
            qT = [qkp.tile([P, S], bf16, tag=f"qT{i}", name=f"qT{i}")
                  for i in range(2)]
            rhs_sb = rhp.tile([P, 4, E_TILES, QQ], bf16)

            # warm-up collective FIRST on the idle gpsimd queue: fires at
            # t~0 on garbage dram data (output unused) so the one-time
            # ncfw setup cost finishes before the first real AllGather.
            wagi = dram.tile([P, 4], bf16, tag="wagi", name="wagi")
            wago = dram.tile([4 * P, 4], bf16, tag="wago", name="wago")
            nc.gpsimd.collective_compute(
                "AllGather",
                mybir.AluOpType.bypass,
                replica_groups=[[0, 1, 2, 3], [4, 5, 6, 7]],
                ins=[wagi.opt()],
                outs=[wago.opt()],
            )

            # ---- DMA issue (all up front, priority order) ----
            # ring A (sync, fast): weights first (tiny), then X chunks in
            # consumption order. ring B (scalar): late-deadline Xq
            # quarters, v q3, wo. (Loading ring B heavily slows the
            # collectives -- SDMA contention -- so it stays light.)
            nc.sync.dma_start(wk[:], WkS)
            nc.sync.dma_start(xk[:, 0:8, :], XkS[:, 0:8, :])
            nc.sync.dma_start(wv[:], WvS)
            nc.sync.dma_start(xv[:, 0:8, :], XvS[:, 0:8, :])
            nc.sync.dma_start(xk[:, 8:16, :], XkS[:, 8:16, :])
            nc.sync.dma_start(xv[:, 8:16, :], XvS[:, 8:16, :])
            nc.sync.dma_start(xv[:, 16:24, :], XvS[:, 16:24, :])
            nc.sync.dma_start(xv[:, 24:32, :], XvS[:, 24:32, :])

            nc.scalar.dma_start(wq[:], WqS)
            nc.scalar.dma_start(bias_c[:], bqko)
            nc.scalar.dma_start(xq[:, 0:8, :], XqS[:, 0:8, :])
            nc.scalar.dma_start(xk[:, 16:24, :], XkS[:, 16:24, :])
            nc.scalar.dma_start(xk[:, 24:32, :], XkS[:, 24:32, :])
            nc.scalar.dma_start(wo[:], WoS)
            nc.scalar.dma_start(xq[:, 8:16, :], XqS[:, 8:16, :])
            nc.scalar.dma_start(xq[:, 16:24, :], XqS[:, 16:24, :])
            nc.scalar.dma_start(xq[:, 24:32, :], XqS[:, 24:32, :])

            # preload the exp table while DMAs stream
            warm = cp.tile([P, 1], f32, tag="warm")
            nc.gpsimd.memset(warm[:], 0.0)
            nc.scalar.activation(warm[:], warm[:], AF.Exp)

            nc.vector.memset(v_sb[:, :, :, Dh], 1.0)

            # one AllGather per quarter for q0-2 (both pairs concatenated
            # on the free axis): the CC core serializes ops from BOTH
            # replica groups, so fewer mid-stream ops keep it unsaturated
            # even on a slow fabric. Quarter 3 is split per-pair: its p0
            # piece gathers under the last block, so only a 128KB op
            # (smallest possible service) remains exposed in the tail.
            ag_in = [dram.tile([P, 2 * QQ], bf16, tag=f"agi{qq}",
                               name=f"agi{qq}") for qq in range(3)]
            ag_out = [dram.tile([4 * P, 2 * QQ], bf16, tag=f"ago{qq}",
                                name=f"ago{qq}") for qq in range(3)]
            ag3_in = [dram.tile([P, QQ], bf16, tag=f"agi3{p}",
                                name=f"agi3{p}") for p in range(2)]
            ag3_out = [dram.tile([4 * P, QQ], bf16, tag=f"ago3{p}",
                                 name=f"ago3{p}") for p in range(2)]

            with (
                tc.tile_pool(name="pss", bufs=2, space="PSUM") as pssp,
                tc.tile_pool(name="psoj0", bufs=1, space="PSUM") as psoj0p,
                tc.tile_pool(name="psoj1", bufs=1, space="PSUM") as psoj1p,
                tc.tile_pool(name="pwork", bufs=2, space="PSUM") as pwp,
            ):
                # ---- chain emitters (projections / O-projection) ----
                def kq_chain(which, p, qb):
                    """Project pair p, seq-quarter qb of k or q."""
                    w, x, dst, b_c = (
                        (wk, xk, kT, bk_c) if which == "k"
                        else (wq, xq, qT, bq_c)
                    )
                    pw = pwp.tile([P, QQ], f32, tag="pw",
                                  name=f"p{which}{p}{qb}")
                    for e in range(E_TILES):
                        nc.tensor.matmul(
                            pw[:],
                            w[:, e, p * P:(p + 1) * P],
                            x[:, qb * 8 + e, :],
                            start=(e == 0),
                            stop=(e == E_TILES - 1),
                        )
                    nc.vector.tensor_scalar_add(
                        dst[p][:, qb * QQ:(qb + 1) * QQ], pw[:],
                        b_c[:, p:p + 1],
                    )

                def v_chain(kt):
                    """Project v key-tile kt into natural [s,h,d] layout."""
                    pw = pwp.tile([P, QQ], f32, tag="pw", name=f"pv{kt}")
                    qb, kk = kt // 4, kt % 4
                    for e in range(E_TILES):
                        nc.tensor.matmul(
                            pw[:, 0:DL],
                            xv[:, qb * 8 + e, kk * P:(kk + 1) * P],
                            wv[:, e, :],
                            start=(e == 0),
                            stop=(e == E_TILES - 1),
                        )
                    nc.vector.tensor_copy(
                        v_sb[:, kt, :, 0:Dh],
                        pw[:, 0:DL].rearrange("p (h d) -> p h d", h=HL),
                    )

                eorder = (0, 2, 4, 6, 1, 3, 5, 7)  # pair-0 ranks first

                def o_chain(qq, dt, pool, tag):
                    """O-projection slice [dt] for seq-quarter qq."""
                    po = pool.tile([P, QQ], f32, tag=tag, name=f"po{qq}{dt}")
                    for ei, e in enumerate(eorder):
                        nc.tensor.matmul(
                            po[:],
                            wo[:, e, dt * P:(dt + 1) * P],
                            rhs_sb[:, qq, e, :],
                            start=(ei == 0),
                            stop=(ei == E_TILES - 1),
                        )
                    ot = resp.tile([P, QQ], bf16, tag="ot", name="ot")
                    nc.scalar.activation(
                        ot[:], po[:], AF.Identity,
                        bias=bo_c[:, dt:dt + 1], scale=1.0,
                    )
                    nc.sync.dma_start(
                        out[dt * P:(dt + 1) * P, qq * QQ:(qq + 1) * QQ],
                        ot[:],
                    )

                # ---- attention: (quarter, pair) blocks x key-tiles ----
                pso_cur = {}

                def scores(u):
                    qq, p, kt = u
                    pss = pssp.tile([P, 2 * QQ], f32, tag="pss", name="pss")
                    for j in range(2):
                        nc.tensor.matmul(
                            pss[:, j * QQ:(j + 1) * QQ],
                            kT[p][64 * j:64 * j + Dh, kt * P:(kt + 1) * P],
                            qT[p][64 * j:64 * j + Dh,
                                  qq * QQ:(qq + 1) * QQ],
                            start=True,
                            stop=True,
                        )
                    ex = expp.tile([P, 2 * QQ], bf16, tag="ex", name="ex")
                    nc.scalar.activation(ex[:], pss[:], AF.Exp)
                    return ex

                def finish(qq, p, pso):
                    # copy PSUM out to SBUF first: the pso banks free after
                    # ~1.5us instead of staying locked for the whole
                    # normalization chain (which would micro-idle the PE at
                    # the block boundary and trip the HAM re-throttle).
                    po_sb = []
                    for j in range(2):
                        ps = nrmp.tile([Dh + 1, QQ], f32, tag=f"posb{j}",
                                       name=f"posb{j}")
                        nc.vector.tensor_copy(ps[:], pso[j][:])
                        po_sb.append(ps)
                    for j in range(2):
                        sums = nrmp.tile([1, QQ], f32, tag="sm", name="sm")
                        nc.sync.dma_start(sums[:], po_sb[j][Dh:Dh + 1, :])
                        recip = nrmp.tile([1, QQ], f32, tag="rc", name="rc")
                        nc.vector.reciprocal_approx_fast(recip[:], sums[:])
                        rcb = nrmp.tile([64, QQ], f32, tag="rcb", name="rcb")
                        nc.sync.dma_start(rcb[0:1, :], recip[:])
                        nc.sync.dma_start(rcb[32:33, :], recip[:])
                        bc = nrmp.tile([64, QQ], f32, tag="bc", name="bc")
                        nc.vector.stream_shuffle(bc[:], rcb[:],
                                                 mask=[0] * 32)
                        ao = aop.tile([Dh, QQ], bf16, tag="ao", name="ao")
                        nc.vector.tensor_mul(ao[:], po_sb[j][0:Dh, :], bc[:])
                        if qq < 3:
                            nc.sync.dma_start(
                                ag_in[qq][64 * j:64 * j + Dh,
                                           p * QQ:(p + 1) * QQ],
                                ao[:],
                            )
                        else:
                            nc.sync.dma_start(
                                ag3_in[p][64 * j:64 * j + Dh, :], ao[:]
                            )
                    if qq < 3 and p == 1:
                        nc.gpsimd.collective_compute(
                            "AllGather",
                            mybir.AluOpType.bypass,
                            replica_groups=[[0, 1, 2, 3], [4, 5, 6, 7]],
                            ins=[ag_in[qq].opt()],
                            outs=[ag_out[qq].opt()],
                        )
                        # prefetch on the gpsimd queue (parked on CC
                        # anyway, so the CC-gated triggers stall nothing):
                        # rank r's [128, 1024] row-block lands in e-slots
                        # 2r (pair 0) and 2r+1 (pair 1).
                        for r in range(4):
                            nc.gpsimd.dma_start(
                                rhs_sb[:, qq, 2 * r:2 * r + 2, :],
                                ag_out[qq][r * P:(r + 1) * P, :],
                            )
                    elif qq == 3:
                        nc.gpsimd.collective_compute(
                            "AllGather",
                            mybir.AluOpType.bypass,
                            replica_groups=[[0, 1, 2, 3], [4, 5, 6, 7]],
                            ins=[ag3_in[p].opt()],
                            outs=[ag3_out[p].opt()],
                        )
                        # the last piece prefetches on the now-idle sync
                        # hwdge ring, which is faster than swdge.
                        eng = nc.gpsimd if p == 0 else nc.sync
                        for r in range(4):
                            eng.dma_start(
                                rhs_sb[:, 3, 2 * r + p, :],
                                ag3_out[p][r * P:(r + 1) * P, :],
                            )

                def pv(u, ex):
                    qq, p, kt = u
                    if kt == 0:
                        pso_cur[(qq, p)] = [
                            psoj0p.tile([Dh + 1, QQ], f32, tag="pso0",
                                        name="pso0"),
                            psoj1p.tile([Dh + 1, QQ], f32, tag="pso1",
                                        name="pso1"),
                        ]
                    pso = pso_cur[(qq, p)]
                    for j in range(2):
                        nc.tensor.matmul(
                            pso[j][:],
                            v_sb[:, kt, 2 * p + j, :],
                            ex[:, j * QQ:(j + 1) * QQ],
                            start=(kt == 0),
                            stop=(kt == KT - 1),
                        )
                    if kt == KT - 1:
                        finish(qq, p, pso)

                units = [
                    (qq, p, kt)
                    for qq in range(4)
                    for p in range(2)
                    for kt in range(KT)
                ]

                # chain ops spliced into the PE stream after unit i.
                # V(kt) at unit kt: maximally late (just before its PV at
                # unit kt+1) so a late v chunk never head-of-line blocks
                # the scores/exp stream. K(p,qb) before scores kt=4qb.
                # O(qq) ~1.5 blocks after AG(qq,p1) fires.
                after_unit = {
                    0: [("v", 0)],
                    1: [("k", 0, 1), ("v", 1)],
                    2: [("v", 2)], 3: [("v", 3)],
                    4: [("k", 1, 0), ("v", 4)],
                    5: [("k", 0, 2), ("v", 5)],
                    6: [("v", 6)], 7: [("v", 7)],
                    8: [("q", 1, 0), ("v", 8)],
                    9: [("k", 0, 3), ("v", 9)],
                    10: [("v", 10)], 11: [("k", 1, 1), ("v", 11)],
                    12: [("v", 12)], 13: [("k", 1, 2), ("v", 13)],
                    14: [("v", 14)],
                    15: [("k", 1, 3), ("v", 15)],
                    28: [("q", 0, 1)],
                    44: [("q", 1, 1)],
                    58: [("q", 0, 2)],
                    76: [("q", 1, 2)],
                    90: [("q", 0, 3)],
                    108: [("q", 1, 3)],
                }

                def emit_chain(op):
                    if op[0] == "v":
                        v_chain(op[1])
                    else:
                        kq_chain(op[0], op[1], op[2])

                # PE clock warm-up: ~90 dummy matmuls on never-written
                # SBUF (no data deps, so they fire at t~0 while the DMAs
                # stream). The HAM un-throttles after ~3.4us of activity,
                # so the real projection chains at ~20us run at full
                # clock instead of the cold 4/8 default.
                junk = cp.tile([P, QQ], bf16, tag="junk")
                nc.vector.memset(junk[:], 0.0)
                wmm = pwp.tile([P, QQ], f32, tag="pw", name="wmm")
                for _ in range(72):
                    nc.tensor.matmul(wmm[:], junk[:, 0:P], junk[:],
                                     start=True, stop=True)

                # prologue: ONLY what unit 0 needs (pair-0 k and q for
                # the first key/query quarters); K(1,0)/Q(1,0) ride the
                # stream (needed by unit 16).
                kq_chain("k", 0, 0)
                kq_chain("q", 0, 0)

                ex_prev = scores(units[0])
                for op in after_unit.get(0, []):
                    emit_chain(op)
                for i in range(1, len(units)):
                    ex = scores(units[i])
                    pv(units[i - 1], ex_prev)
                    ex_prev = ex
                    for op in after_unit.get(i, []):
                        emit_chain(op)
                pv(units[-1], ex_prev)

                # tail: ALL O-projections run here, ordered so the PE is
                # never idle during the last gather. Quarter 3's pair-0
                # halves first (their AG completed under block 7), then
                # quarters 0-2 (AGs long done) fill the final 128KB AG's
                # service window, then quarter 3's pair-1 halves.
                po3 = [pwp.tile([P, QQ], f32, tag="pw", name=f"po3{dt}")
                       for dt in range(2)]
                for dt in range(2):
                    for ei, e in enumerate((0, 2, 4, 6)):
                        nc.tensor.matmul(
                            po3[dt][:],
                            wo[:, e, dt * P:(dt + 1) * P],
                            rhs_sb[:, 3, e, :],
                            start=(ei == 0),
                            stop=False,
                        )
                for qq in range(3):
                    o_chain(qq, 0, psoj0p, "pso0")
                    o_chain(qq, 1, psoj1p, "pso1")
                for dt in range(2):
                    for ei, e in enumerate((1, 3, 5, 7)):
                        nc.tensor.matmul(
                            po3[dt][:],
                            wo[:, e, dt * P:(dt + 1) * P],
                            rhs_sb[:, 3, e, :],
                            start=False,
                            stop=(ei == 3),
                        )
                for dt in range(2):
                    ot = resp.tile([P, QQ], bf16, tag="ot", name="ot")
                    nc.scalar.activation(
                        ot[:], po3[dt][:], AF.Identity,
                        bias=bo_c[:, dt:dt + 1], scale=1.0,
                    )
                    nc.sync.dma_start(
                        out[dt * P:(dt + 1) * P, 3 * QQ:4 * QQ], ot[:],
                    )

    nc.compile()
    return nc


def kernel(**inputs):
    Q = np.asarray(inputs["Q"], dtype=np.float32)
    K = np.asarray(inputs["K"], dtype=np.float32)
    V = np.asarray(inputs["V"], dtype=np.float32)
    Wq = np.asarray(inputs["Wq"], dtype=np.float32)
    Wk = np.asarray(inputs["Wk"], dtype=np.float32)
    Wv = np.asarray(inputs["Wv"], dtype=np.float32)
    Wo = np.asarray(inputs["Wo"], dtype=np.float32)
    bq = np.asarray(inputs["bq"], dtype=np.float32)
    bk = np.asarray(inputs["bk"], dtype=np.float32)
    bv = np.asarray(inputs["bv"], dtype=np.float32)
    bo = np.asarray(inputs["bo"], dtype=np.float32)

    nc = _build()

    def xswz(x):  # [S, D] -> [128, qb*8+e, 512] (contiguous quarter chunks)
        t = x.T.reshape(E_TILES, P, 4, QQ).transpose(1, 2, 0, 3)
        return np.ascontiguousarray(t.reshape(P, 32, QQ)).astype(
            ml_dtypes.bfloat16)

    XT = {
        b: {
            "XvS": xswz(V[b]),
            "XkS": xswz(K[b]),
            "XqS": xswz(Q[b]),
        }
        for b in range(B)
    }

    # Heads within a group are laid out (0, 2, 1, 3) so each projected tile
    # holds a head pair spanning both PE row-group halves. W tensors are
    # pre-swizzled to the SBUF layout [128, e-tile, 256] for contiguous DMA.
    horder = (0, 2, 1, 3)

    def swz(w):  # [256 out-perm, 1024 in] -> [128, 8, 256]
        return np.ascontiguousarray(
            w.T.reshape(E_TILES, P, DL).transpose(1, 0, 2)
        ).astype(ml_dtypes.bfloat16)

    # Wo columns per e-tile follow the gathered attention rows:
    # e = (rank r, pair p); rows = heads horder[2p], horder[2p+1] of rank r.
    colperm = []
    for r in range(4):
        for p_ in range(2):
            for j in range(2):
                hl = horder[2 * p_ + j]
                colperm.extend(range(DL * r + 64 * hl, DL * r + 64 * hl + 64))
    colperm = np.array(colperm)

    Wslices = {}
    for g in range(4):
        idx = np.concatenate(
            [
                np.arange(DL * g + 64 * hl, DL * g + 64 * hl + 64)
                for hl in horder
            ]
        )
        rows = slice(DL * g, DL * (g + 1))
        Wslices[g] = {
            "WvS": swz(Wv[idx]),
            "WkS": swz(Wk[idx]),
            "WqS": swz(Wq[idx] * 0.125),
            "WoS": swz(Wo[rows][:, colperm]),
            "bqko": np.ascontiguousarray(
                np.stack(
                    [
                        bq[idx] * 0.125,
                        bk[idx],
                        bo[rows] + Wo[rows] @ bv,
                    ],
                    axis=0,
                )
                .reshape(3, 2, P)
                .transpose(2, 0, 1)
                .reshape(P, 6)
            ).astype(np.float32),
        }

    in_maps = []
    for c in range(N_CORES):
        b, g = c // 4, c % 4
        m = dict(XT[b])
        m.update(Wslices[g])
        in_maps.append(m)

    trace_cores = (
        list(range(N_CORES)) if os.environ.get("TRACE_ALL") else None
    )
    res = bass_utils.run_bass_kernel_spmd(
        nc, in_maps, core_ids=list(range(N_CORES)), trace=TRACE,
        trace_cores=trace_cores,
    )

    full = np.empty((B, S, D), dtype=np.float32)
    for c in range(N_CORES):
        b, g = c // 4, c % 4
        full[b, :, DL * g:DL * (g + 1)] = (
            res.results[c]["out"].astype(np.float32).T
        )
    if TRACE:
        kernel.last_result = res
    return full


kernel.last_result = None


# revision 47
# speedup vs baseline: 1.1919x; 1.0015x over previous
"""Multi-head attention (B=2, S=2048, D=1024, H=16) on 8 trn2 NeuronCores.

Sharding: core c -> batch b = c//4, head-group g = c%4 (4 heads each).
Tensor-parallel on heads: each core projects q/k/v for its 4 heads only
(column-sharded W_q/W_k/W_v), runs full-sequence attention for those heads,
all-gathers the per-head attention outputs within its 4-core batch group,
and computes a 256-column slice of the final W_o projection. The host
reassembles the [2, 2048, 1024] output from the 8 per-core [256, 2048]
(transposed) slices.

v2 structure (vs the ~287us phase-separated predecessor; ~275us in the
same fabric conditions where v1 measures ~300):
- The exp stream (128x [128,1024] ACTIVATEs, 142.5us) now starts at
  ~35us instead of ~82us: X is host-swizzled to [128, qb*8+e, 512] so
  each seq-quarter chunk is one contiguous-8KB-per-partition DMA, and
  the projection chains (8 e-tile matmuls + DVE eviction each) chase
  the chunks, spliced between attention units in PE emission order.
- ~90 dummy matmuls on never-written SBUF fire at t~0 to hold the HAM
  clock-gate open, so the first real chains run at full clock.
- Attention unit order is quarter-outer / pair-inner. One AllGather
  per quarter for q0-2 (pairs concatenated on the free axis -- the CC
  core serializes ops from BOTH replica groups, so fewer mid-stream
  ops keep it unsaturated); quarter 3 is split per-pair so its p0
  piece gathers under the last block and only a 128KB op remains
  exposed in the tail.
- finish() copies the PV PSUM to SBUF first (banks free in ~1.5us
  instead of locked behind the whole normalization chain), which
  killed a HAM re-throttle oscillation at every block boundary.
- ALL O-projection chains run in the tail: quarters 0-2 (AGs long
  done) fill the last AG's service window; quarter 3's pair-0 halves
  run before it, pair-1 halves after. Output is written bf16.
- PSUM: scores ping-pong 2x[128,1024] (4 banks) + PV accumulators
  2x[65,512] single-generation (2 banks) + a 2-bank ping-pong pool
  for projection chains / warm-up / q3 O-chains; q0-2 O-chains reuse
  the PV pools.
- Everything else (head-pair packing, ones-column softmax sums, DVE
  normalization, bias foldings, gpsimd AG prefetch, warm-up collective
  at t~0) is inherited from v1.
"""

import os

import ml_dtypes
import numpy as np

import concourse.bacc as bacc
import concourse.mybir as mybir
import concourse.tile as tile
from concourse import bass_utils

B, S, D, H = 2, 2048, 1024, 16
Dh = D // H  # 64
N_CORES = 8
HL = H // 4  # heads per core (4)
DL = HL * Dh  # local head dims (256)
P = 128
E_TILES = D // P  # 8
KT = S // P  # 16 key tiles
QQ = 512  # s_q quarter block

f32 = mybir.dt.float32
bf16 = mybir.dt.bfloat16
AF = mybir.ActivationFunctionType

TRACE = False  # test harness sets kernel.TRACE = True for profiling


def _build():
    nc = bacc.Bacc("TRN2", target_bir_lowering=False, debug=False,
                   num_devices=N_CORES)

    # X swizzled [128, qb*8+e, 512]: one seq-quarter = contiguous 8KB/part
    XvS = nc.dram_tensor("XvS", [P, 32, QQ], bf16, kind="ExternalInput").ap()
    XkS = nc.dram_tensor("XkS", [P, 32, QQ], bf16, kind="ExternalInput").ap()
    XqS = nc.dram_tensor("XqS", [P, 32, QQ], bf16, kind="ExternalInput").ap()
    WvS = nc.dram_tensor("WvS", [P, E_TILES, DL], bf16,
                         kind="ExternalInput").ap()
    WkS = nc.dram_tensor("WkS", [P, E_TILES, DL], bf16,
                         kind="ExternalInput").ap()
    WqS = nc.dram_tensor("WqS", [P, E_TILES, DL], bf16,
                         kind="ExternalInput").ap()
    WoS = nc.dram_tensor("WoS", [P, E_TILES, DL], bf16,
                         kind="ExternalInput").ap()
    bqko = nc.dram_tensor("bqko", [P, 6], f32, kind="ExternalInput").ap()
    out = nc.dram_tensor("out", [DL, S], bf16, kind="ExternalOutput").ap()

    with tile.TileContext(nc) as tc:
        with (
            tc.tile_pool(name="const", bufs=1) as cp,
            tc.tile_pool(name="qk", bufs=1) as qkp,
            tc.tile_pool(name="vt", bufs=1) as vtp,
            tc.tile_pool(name="xs", bufs=1) as xsp,
            tc.tile_pool(name="rhs", bufs=1) as rhp,
            tc.tile_pool(name="exp", bufs=6) as expp,
            tc.tile_pool(name="nrm", bufs=1) as nrmp,
            tc.tile_pool(name="ao", bufs=4) as aop,
            tc.tile_pool(name="res", bufs=2) as resp,
            tc.tile_pool(name="dram", bufs=1, space="DRAM") as dram,
        ):
            # ---- SBUF tiles ----
            wv = cp.tile([P, E_TILES, DL], bf16, tag="wv")
            wk = cp.tile([P, E_TILES, DL], bf16, tag="wk")
            wq = cp.tile([P, E_TILES, DL], bf16, tag="wq")
            wo = cp.tile([P, E_TILES, DL], bf16, tag="wo")
            bias_c = cp.tile([P, 6], f32, tag="bias")
            bq_c = bias_c[:, 0:2]
            bk_c = bias_c[:, 2:4]
            bo_c = bias_c[:, 4:6]

            xv = xsp.tile([P, 32, QQ], bf16, tag="xv", name="xv")
            xk = xsp.tile([P, 32, QQ], bf16, tag="xk", name="xk")
            xq = xsp.tile([P, 32, QQ], bf16, tag="xq", name="xq")

            v_sb = vtp.tile([P, KT, HL, Dh + 1], bf16)
            kT = [qkp.tile([P, S], bf16, tag=f"kT{i}", name=f"kT{i}")
                  for i in range(2)]
            qT = [qkp.tile([P, S], bf16, tag=f"qT{i}", name=f"qT{i}")
                  for i in range(2)]
            rhs_sb = rhp.tile([P, 4, E_TILES, QQ], bf16)

            # warm-up collective FIRST on the idle gpsimd queue: fires at
            # t~0 on garbage dram data (output unused) so the one-time
            # ncfw setup cost finishes before the first real AllGather.
            wagi = dram.tile([P, 4], bf16, tag="wagi", name="wagi")
            wago = dram.tile([4 * P, 4], bf16, tag="wago", name="wago")
            nc.gpsimd.collective_compute(
                "AllGather",
                mybir.AluOpType.bypass,
                replica_groups=[[0, 1, 2, 3], [4, 5, 6, 7]],
                ins=[wagi.opt()],
                outs=[wago.opt()],
            )

            # ---- DMA issue (all up front, priority order) ----
            # ring A (sync, fast): weights first (tiny), then X chunks in
            # consumption order. ring B (scalar): late-deadline Xq
            # quarters, v q3, wo. (Loading ring B heavily slows the
            # collectives -- SDMA contention -- so it stays light.)
            nc.sync.dma_start(wk[:], WkS)
            nc.sync.dma_start(xk[:, 0:8, :], XkS[:, 0:8, :])
            nc.sync.dma_start(wv[:], WvS)
            nc.sync.dma_start(xv[:, 0:8, :], XvS[:, 0:8, :])
            nc.sync.dma_start(xk[:, 8:16, :], XkS[:, 8:16, :])
            nc.sync.dma_start(xv[:, 8:16, :], XvS[:, 8:16, :])
            nc.sync.dma_start(xv[:, 16:24, :], XvS[:, 16:24, :])
            nc.sync.dma_start(xv[:, 24:32, :], XvS[:, 24:32, :])

            nc.scalar.dma_start(wq[:], WqS)
            nc.scalar.dma_start(bias_c[:], bqko)
            nc.scalar.dma_start(xq[:, 0:8, :], XqS[:, 0:8, :])
            nc.scalar.dma_start(xk[:, 16:24, :], XkS[:, 16:24, :])
            nc.scalar.dma_start(xk[:, 24:32, :], XkS[:, 24:32, :])
            nc.scalar.dma_start(wo[:], WoS)
            nc.scalar.dma_start(xq[:, 8:16, :], XqS[:, 8:16, :])
            nc.scalar.dma_start(xq[:, 16:24, :], XqS[:, 16:24, :])
            nc.scalar.dma_start(xq[:, 24:32, :], XqS[:, 24:32, :])

            # preload the exp table while DMAs stream
            warm = cp.tile([P, 1], f32, tag="warm")
            nc.gpsimd.memset(warm[:], 0.0)
            nc.scalar.activation(warm[:], warm[:], AF.Exp)

            nc.vector.memset(v_sb[:, :, :, Dh], 1.0)

            # one AllGather per quarter for q0-2 (both pairs concatenated
            # on the free axis): the CC core serializes ops from BOTH
            # replica groups, so fewer mid-stream ops keep it unsaturated
            # even on a slow fabric. Quarter 3 is split per-pair: its p0
            # piece gathers under the last block, so only a 128KB op
            # (smallest possible service) remains exposed in the tail.
            ag_in = [dram.tile([P, 2 * QQ], bf16, tag=f"agi{qq}",
                               name=f"agi{qq}") for qq in range(3)]
            ag_out = [dram.tile([4 * P, 2 * QQ], bf16, tag=f"ago{qq}",
                                name=f"ago{qq}") for qq in range(3)]
            ag3_in = [dram.tile([P, QQ], bf16, tag=f"agi3{p}",
                                name=f"agi3{p}") for p in range(2)]
            ag3_out = [dram.tile([4 * P, QQ], bf16, tag=f"ago3{p}",
                                 name=f"ago3{p}") for p in range(2)]

            with (
                tc.tile_pool(name="pss", bufs=2, space="PSUM") as pssp,
                tc.tile_pool(name="psoj0", bufs=1, space="PSUM") as psoj0p,
                tc.tile_pool(name="psoj1", bufs=1, space="PSUM") as psoj1p,
                tc.tile_pool(name="pwork", bufs=2, space="PSUM") as pwp,
            ):
                # ---- chain emitters (projections / O-projection) ----
                def kq_chain(which, p, qb):
                    """Project pair p, seq-quarter qb of k or q."""
                    w, x, dst, b_c = (
                        (wk, xk, kT, bk_c) if which == "k"
                        else (wq, xq, qT, bq_c)
                    )
                    pw = pwp.tile([P, QQ], f32, tag="pw",
                                  name=f"p{which}{p}{qb}")
                    for e in range(E_TILES):
                        nc.tensor.matmul(
                            pw[:],
                            w[:, e, p * P:(p + 1) * P],
                            x[:, qb * 8 + e, :],
                            start=(e == 0),
                            stop=(e == E_TILES - 1),
                        )
                    nc.vector.tensor_scalar_add(
                        dst[p][:, qb * QQ:(qb + 1) * QQ], pw[:],
                        b_c[:, p:p + 1],
                    )

                def v_chain(kt):
                    """Project v key-tile kt into natural [s,h,d] layout."""
                    pw = pwp.tile([P, QQ], f32, tag="pw", name=f"pv{kt}")
                    qb, kk = kt // 4, kt % 4
                    for e in range(E_TILES):
                        nc.tensor.matmul(
                            pw[:, 0:DL],
                            xv[:, qb * 8 + e, kk * P:(kk + 1) * P],
                            wv[:, e, :],
                            start=(e == 0),
                            stop=(e == E_TILES - 1),
                        )
                    nc.vector.tensor_copy(
                        v_sb[:, kt, :, 0:Dh],
                        pw[:, 0:DL].rearrange("p (h d) -> p h d", h=HL),
                    )

                eorder = (0, 2, 4, 6, 1, 3, 5, 7)  # pair-0 ranks first

                def o_chain(qq, dt, pool, tag):
                    """O-projection slice [dt] for seq-quarter qq."""
                    po = pool.tile([P, QQ], f32, tag=tag, name=f"po{qq}{dt}")
                    for ei, e in enumerate(eorder):
                        nc.tensor.matmul(
                            po[:],
                            wo[:, e, dt * P:(dt + 1) * P],
                            rhs_sb[:, qq, e, :],
                            start=(ei == 0),
                            stop=(ei == E_TILES - 1),
                        )
                    ot = resp.tile([P, QQ], bf16, tag="ot", name="ot")
                    nc.scalar.activation(
                        ot[:], po[:], AF.Identity,
                        bias=bo_c[:, dt:dt + 1], scale=1.0,
                    )
                    nc.sync.dma_start(
                        out[dt * P:(dt + 1) * P, qq * QQ:(qq + 1) * QQ],
                        ot[:],
                    )

                # ---- attention: (quarter, pair) blocks x key-tiles ----
                pso_cur = {}

                def scores(u):
                    qq, p, kt = u
                    pss = pssp.tile([P, 2 * QQ], f32, tag="pss", name="pss")
                    for j in range(2):
                        nc.tensor.matmul(
                            pss[:, j * QQ:(j + 1) * QQ],
                            kT[p][64 * j:64 * j + Dh, kt * P:(kt + 1) * P],
                            qT[p][64 * j:64 * j + Dh,
                                  qq * QQ:(qq + 1) * QQ],
                            start=True,
                            stop=True,
                        )
                    ex = expp.tile([P, 2 * QQ], bf16, tag="ex", name="ex")
                    nc.scalar.activation(ex[:], pss[:], AF.Exp)
                    return ex

                def finish(qq, p, pso):
                    # copy PSUM out to SBUF first: the pso banks free after
                    # ~1.5us instead of staying locked for the whole
                    # normalization chain (which would micro-idle the PE at
                    # the block boundary and trip the HAM re-throttle).
                    po_sb = []
                    for j in range(2):
                        ps = nrmp.tile([Dh + 1, QQ], f32, tag=f"posb{j}",
                                       name=f"posb{j}")
                        nc.vector.tensor_copy(ps[:], pso[j][:])
                        po_sb.append(ps)
                    for j in range(2):
                        sums = nrmp.tile([1, QQ], f32, tag="sm", name="sm")
                        nc.sync.dma_start(sums[:], po_sb[j][Dh:Dh + 1, :])
                        recip = nrmp.tile([1, QQ], f32, tag="rc", name="rc")
                        nc.vector.reciprocal_approx_fast(recip[:], sums[:])
                        rcb = nrmp.tile([64, QQ], f32, tag="rcb", name="rcb")
                        nc.sync.dma_start(rcb[0:1, :], recip[:])
                        nc.sync.dma_start(rcb[32:33, :], recip[:])
                        bc = nrmp.tile([64, QQ], f32, tag="bc", name="bc")
                        nc.vector.stream_shuffle(bc[:], rcb[:],
                                                 mask=[0] * 32)
                        ao = aop.tile([Dh, QQ], bf16, tag="ao", name="ao")
                        nc.vector.tensor_mul(ao[:], po_sb[j][0:Dh, :], bc[:])
                        if qq < 3:
                            nc.sync.dma_start(
                                ag_in[qq][64 * j:64 * j + Dh,
                                           p * QQ:(p + 1) * QQ],
                                ao[:],
                            )
                        else:
                            nc.sync.dma_start(
                                ag3_in[p][64 * j:64 * j + Dh, :], ao[:]
                            )
                    if qq < 3 and p == 1:
                        nc.gpsimd.collective_compute(
                            "AllGather",
                            mybir.AluOpType.bypass,
                            replica_groups=[[0, 1, 2, 3], [4, 5, 6, 7]],
                            ins=[ag_in[qq].opt()],
                            outs=[ag_out[qq].opt()],
                        )
                        # prefetch on the gpsimd queue (parked on CC
                        # anyway, so the CC-gated triggers stall nothing):
                        # rank r's [128, 1024] row-block lands in e-slots
                        # 2r (pair 0) and 2r+1 (pair 1).
                        for r in range(4):
                            nc.gpsimd.dma_start(
                                rhs_sb[:, qq, 2 * r:2 * r + 2, :],
                                ag_out[qq][r * P:(r + 1) * P, :],
                            )
                    elif qq == 3:
                        nc.gpsimd.collective_compute(
                            "AllGather",
                            mybir.AluOpType.bypass,
                            replica_groups=[[0, 1, 2, 3], [4, 5, 6, 7]],
                            ins=[ag3_in[p].opt()],
                            outs=[ag3_out[p].opt()],
                        )
                        # the last piece prefetches on the now-idle sync
                        # hwdge ring, which is faster than swdge.
                        eng = nc.gpsimd if p == 0 else nc.sync
                        for r in range(4):
                            eng.dma_start(
                                rhs_sb[:, 3, 2 * r + p, :],
                                ag3_out[p][r * P:(r + 1) * P, :],
                            )

                def pv(u, ex):
                    qq, p, kt = u
                    if kt == 0:
                        pso_cur[(qq, p)] = [
                            psoj0p.tile([Dh + 1, QQ], f32, tag="pso0",
                                        name="pso0"),
                            psoj1p.tile([Dh + 1, QQ], f32, tag="pso1",
                                        name="pso1"),
                        ]
                    pso = pso_cur[(qq, p)]
                    for j in range(2):
                        nc.tensor.matmul(
                            pso[j][:],
                            v_sb[:, kt, 2 * p + j, :],
                            ex[:, j * QQ:(j + 1) * QQ],
                            start=(kt == 0),
                            stop=(kt == KT - 1),
                        )
                    if kt == KT - 1:
                        finish(qq, p, pso)

                units = [
                    (qq, p, kt)
                    for qq in range(4)
                    for p in range(2)
                    for kt in range(KT)
                ]

                # chain ops spliced into the PE stream after unit i.
                # V(kt) at unit kt: maximally late (just before its PV at
                # unit kt+1) so a late v chunk never head-of-line blocks
                # the scores/exp stream. K(p,qb) before scores kt=4qb.
                # O(qq) ~1.5 blocks after AG(qq,p1) fires.
                after_unit = {
                    0: [("v", 0), ("v", 1)],
                    1: [("k", 0, 1), ("v", 2)],
                    2: [("v", 3)], 3: [("v", 4)],
                    4: [("k", 1, 0), ("v", 5)],
                    5: [("k", 0, 2), ("v", 6)],
                    6: [("v", 7)], 7: [("v", 8)],
                    8: [("q", 1, 0), ("v", 9)],
                    9: [("k", 0, 3), ("v", 10)],
                    10: [("v", 11)], 11: [("k", 1, 1), ("v", 12)],
                    12: [("v", 13)], 13: [("k", 1, 2), ("v", 14)],
                    14: [("v", 15)],
                    15: [("k", 1, 3)],
                    28: [("q", 0, 1)],
                    44: [("q", 1, 1)],
                    58: [("q", 0, 2)],
                    76: [("q", 1, 2)],
                    90: [("q", 0, 3)],
                    108: [("q", 1, 3)],
                }

                def emit_chain(op):
                    if op[0] == "v":
                        v_chain(op[1])
                    else:
                        kq_chain(op[0], op[1], op[2])

                # PE clock warm-up: ~90 dummy matmuls on never-written
                # SBUF (no data deps, so they fire at t~0 while the DMAs
                # stream). The HAM un-throttles after ~3.4us of activity,
                # so the real projection chains at ~20us run at full
                # clock instead of the cold 4/8 default.
                junk = cp.tile([P, QQ], bf16, tag="junk")
                nc.vector.memset(junk[:], 0.0)
                wmm = pwp.tile([P, QQ], f32, tag="pw", name="wmm")
                for _ in range(72):
                    nc.tensor.matmul(wmm[:], junk[:, 0:P], junk[:],
                                     start=True, stop=True)

                # prologue: ONLY what unit 0 needs (pair-0 k and q for
                # the first key/query quarters); K(1,0)/Q(1,0) ride the
                # stream (needed by unit 16).
                kq_chain("k", 0, 0)
                kq_chain("q", 0, 0)

                ex_prev = scores(units[0])
                for op in after_unit.get(0, []):
                    emit_chain(op)
                for i in range(1, len(units)):
                    ex = scores(units[i])
                    pv(units[i - 1], ex_prev)
                    ex_prev = ex
                    for op in after_unit.get(i, []):
                        emit_chain(op)
                pv(units[-1], ex_prev)

                # tail: ALL O-projections run here, ordered so the PE is
                # never idle during the last gather. Quarter 3's pair-0
                # halves first (their AG completed under block 7), then
                # quarters 0-2 (AGs long done) fill the final 128KB AG's
                # service window, then quarter 3's pair-1 halves.
                po3 = [pwp.tile([P, QQ], f32, tag="pw", name=f"po3{dt}")
                       for dt in range(2)]
                for dt in range(2):
                    for ei, e in enumerate((0, 2, 4, 6)):
                        nc.tensor.matmul(
                            po3[dt][:],
                            wo[:, e, dt * P:(dt + 1) * P],
                            rhs_sb[:, 3, e, :],
                            start=(ei == 0),
                            stop=False,
                        )
                for qq in range(3):
                    o_chain(qq, 0, psoj0p, "pso0")
                    o_chain(qq, 1, psoj1p, "pso1")
                for dt in range(2):
                    for ei, e in enumerate((1, 3, 5, 7)):
                        nc.tensor.matmul(
                            po3[dt][:],
                            wo[:, e, dt * P:(dt + 1) * P],
                            rhs_sb[:, 3, e, :],
                            start=False,
                            stop=(ei == 3),
                        )
                for dt in range(2):
                    ot = resp.tile([P, QQ], bf16, tag="ot", name="ot")
                    nc.scalar.activation(
                        ot[:], po3[dt][:], AF.Identity,
                        bias=bo_c[:, dt:dt + 1], scale=1.0,
                    )
                    nc.sync.dma_start(
                        out[dt * P:(dt + 1) * P, 3 * QQ:4 * QQ], ot[:],
                    )

    nc.compile()
    return nc


def kernel(**inputs):
    Q = np.asarray(inputs["Q"], dtype=np.float32)
    K = np.asarray(inputs["K"], dtype=np.float32)
    V = np.asarray(inputs["V"], dtype=np.float32)
    Wq = np.asarray(inputs["Wq"], dtype=np.float32)
    Wk = np.asarray(inputs["Wk"], dtype=np.float32)
    Wv = np.asarray(inputs["Wv"], dtype=np.float32)
    Wo = np.asarray(inputs["Wo"], dtype=np.float32)
    bq = np.asarray(inputs["bq"], dtype=np.float32)
    bk = np.asarray(inputs["bk"], dtype=np.float32)
    bv = np.asarray(inputs["bv"], dtype=np.float32)
    bo = np.asarray(inputs["bo"], dtype=np.float32)

    nc = _build()

    def xswz(x):  # [S, D] -> [128, qb*8+e, 512] (contiguous quarter chunks)
        t = x.T.reshape(E_TILES, P, 4, QQ).transpose(1, 2, 0, 3)
        return np.ascontiguousarray(t.reshape(P, 32, QQ)).astype(
            ml_dtypes.bfloat16)

    XT = {
        b: {
            "XvS": xswz(V[b]),
            "XkS": xswz(K[b]),
            "XqS": xswz(Q[b]),
        }
        for b in range(B)
    }

    # Heads within a group are laid out (0, 2, 1, 3) so each projected tile
    # holds a head pair spanning both PE row-group halves. W tensors are
    # pre-swizzled to the SBUF layout [128, e-tile, 256] for contiguous DMA.
    horder = (0, 2, 1, 3)

    def swz(w):  # [256 out-perm, 1024 in] -> [128, 8, 256]
        return np.ascontiguousarray(
            w.T.reshape(E_TILES, P, DL).transpose(1, 0, 2)
        ).astype(ml_dtypes.bfloat16)

    # Wo columns per e-tile follow the gathered attention rows:
    # e = (rank r, pair p); rows = heads horder[2p], horder[2p+1] of rank r.
    colperm = []
    for r in range(4):
        for p_ in range(2):
            for j in range(2):
                hl = horder[2 * p_ + j]
                colperm.extend(range(DL * r + 64 * hl, DL * r + 64 * hl + 64))
    colperm = np.array(colperm)

    Wslices = {}
    for g in range(4):
        idx = np.concatenate(
            [
                np.arange(DL * g + 64 * hl, DL * g + 64 * hl + 64)
                for hl in horder
            ]
        )
        rows = slice(DL * g, DL * (g + 1))
        Wslices[g] = {
            "WvS": swz(Wv[idx]),
            "WkS": swz(Wk[idx]),
            "WqS": swz(Wq[idx] * 0.125),
            "WoS": swz(Wo[rows][:, colperm]),
            "bqko": np.ascontiguousarray(
                np.stack(
                    [
                        bq[idx] * 0.125,
                        bk[idx],
                        bo[rows] + Wo[rows] @ bv,
                    ],
                    axis=0,
                )
                .reshape(3, 2, P)
                .transpose(2, 0, 1)
                .reshape(P, 6)
            ).astype(np.float32),
        }

    in_maps = []
    for c in range(N_CORES):
        b, g = c // 4, c % 4
        m = dict(XT[b])
        m.update(Wslices[g])
        in_maps.append(m)

    trace_cores = (
        list(range(N_CORES)) if os.environ.get("TRACE_ALL") else None
    )
    res = bass_utils.run_bass_kernel_spmd(
        nc, in_maps, core_ids=list(range(N_CORES)), trace=TRACE,
        trace_cores=trace_cores,
    )

    full = np.empty((B, S, D), dtype=np.float32)
    for c in range(N_CORES):
        b, g = c // 4, c % 4
        full[b, :, DL * g:DL * (g + 1)] = (
            res.results[c]["out"].astype(np.float32).T
        )
    if TRACE:
        kernel.last_result = res
    return full


kernel.last_result = None
